# revision 1
# baseline (speedup 1.0000x reference)
"""Trainium2 Bass kernel for a minimal Mamba layer (B=2, L=2048, d_model=1024,
d_inner=2048, d_state=16, d_conv=4, dt_rank=64) on 8 NeuronCores.

Sharding: core = (batch, d_inner-quarter).  Cores 0-3 handle batch 0, cores
4-7 batch 1; within a batch group each core owns 512 d_inner channels.

Two SPMD kernels with a tiny host exchange between them:
  A: in_proj (own rows) + causal depthwise conv (as 4 PSUM-accumulated
     diagonal matmuls) + silu + x_proj partial (own-channel contraction).
  host: sum the 4 partial dbc's per batch (96x2048 each), build broadcast
     tiles for B/C rows.
  B: dt_proj + softplus, then per (state, ch-block): dA = exp(A*delta) on
     ScalarE, Bu on VectorE, the SSM recurrence via the hardware
     tensor_tensor_scan, y accumulation, gating, out_proj partial.
  host: sum the 4 partial outputs per batch.
"""

import sys

if "/opt/trn_rl_repo" not in sys.path:
    sys.path.insert(0, "/opt/trn_rl_repo")

import numpy as np
import ml_dtypes

import concourse.bass as bass
from concourse import bacc, mybir
from concourse.bass_utils import run_bass_kernel_spmd
from concourse.tile import TileContext

F32 = mybir.dt.float32
BF16 = mybir.dt.bfloat16
AF = mybir.ActivationFunctionType
OP = mybir.AluOpType

D_MODEL = 1024
D_STATE = 16
D_CONV = 4
D_INNER = 2048
DT_RANK = 64
B = 2
L = 2048
NCORES = 8
CH = D_INNER // 4          # 512 channels per core
NCB = CH // 128            # 4 channel blocks of 128
NT = L // 512              # 4 token tiles of 512
KM = D_MODEL // 128        # 8 k tiles for in_proj

_CACHE = {}


def _build_a():
    nc = bacc.Bacc("TRN2", target_bir_lowering=False, debug=False,
                   num_devices=NCORES)
    xT = nc.dram_tensor("xT", [D_MODEL, L], BF16, kind="ExternalInput").ap()
    w1t = nc.dram_tensor("w1t", [D_MODEL, 2 * CH], BF16, kind="ExternalInput").ap()
    convdiag = nc.dram_tensor("convdiag", [D_CONV * NCB * 128, 128], BF16,
                              kind="ExternalInput").ap()
    convb = nc.dram_tensor("convb", [128, NCB], F32, kind="ExternalInput").ap()
    wxpT = nc.dram_tensor("wxpT", [CH, 96], BF16, kind="ExternalInput").ap()

    xc_out = nc.dram_tensor("xc", [CH, L], BF16, kind="ExternalOutput").ap()
    sres_out = nc.dram_tensor("sres", [CH, L], BF16, kind="ExternalOutput").ap()
    dbc_out = nc.dram_tensor("dbc", [96, L], F32, kind="ExternalOutput").ap()

    with TileContext(nc) as tc:
        with (
            tc.tile_pool(name="const", bufs=1) as const,
            tc.tile_pool(name="psum", bufs=4, space="PSUM") as psum,
            tc.tile_pool(name="work", bufs=3) as work,
        ):
            xT_t, w1_t = [], []
            for k in range(KM):
                t = const.tile([128, L], BF16, tag=f"xT{k}")
                nc.sync.dma_start(out=t[:], in_=xT[k * 128:(k + 1) * 128, :])
                xT_t.append(t)
                t = const.tile([128, 2 * CH], BF16, tag=f"w1{k}", name=f"w1{k}")
                nc.sync.dma_start(out=t[:], in_=w1t[k * 128:(k + 1) * 128, :])
                w1_t.append(t)
            cdiag = []
            for tap in range(D_CONV):
                row = []
                for cb in range(NCB):
                    t = const.tile([128, 128], BF16, tag=f"cd{tap}_{cb}")
                    off = (tap * NCB + cb) * 128
                    nc.sync.dma_start(out=t[:], in_=convdiag[off:off + 128, :])
                    row.append(t)
                cdiag.append(row)
            cb_t = const.tile([128, NCB], F32, tag="convb")
            nc.sync.dma_start(out=cb_t[:], in_=convb[:])
            wxp_t = []
            for kc in range(NCB):
                t = const.tile([128, 96], BF16, tag=f"wxp{kc}")
                nc.sync.dma_start(out=t[:], in_=wxpT[kc * 128:(kc + 1) * 128, :])
                wxp_t.append(t)

            # xi (post in_proj, pre conv): padded with 3 zero columns in front
            xi_pad = []
            for cb in range(NCB):
                t = const.tile([128, L + D_CONV - 1], BF16, tag=f"xip{cb}")
                nc.vector.memset(t[:, 0:D_CONV - 1], 0.0)
                xi_pad.append(t)
            xc_t = [const.tile([128, L], BF16, tag=f"xc{cb}", name=f"xc{cb}") for cb in range(NCB)]

            # ---- in_proj: out rows 0..511 = xi slice, 512..1023 = res slice
            for n in range(NT):
                for m in range(2 * NCB):
                    pt = psum.tile([128, 512], F32, tag="mm")
                    for k in range(KM):
                        nc.tensor.matmul(
                            pt[:], w1_t[k][:, m * 128:(m + 1) * 128],
                            xT_t[k][:, n * 512:(n + 1) * 512],
                            start=(k == 0), stop=(k == KM - 1))
                    if m < NCB:
                        nc.scalar.activation(
                            xi_pad[m][:, D_CONV - 1 + n * 512:
                                      D_CONV - 1 + (n + 1) * 512],
                            pt[:], AF.Copy)
                    else:
                        st = work.tile([128, 512], BF16, tag="sres")
                        nc.scalar.activation(st[:], pt[:], AF.Silu)
                        nc.sync.dma_start(
                            out=sres_out[(m - NCB) * 128:(m - NCB + 1) * 128,
                                         n * 512:(n + 1) * 512],
                            in_=st[:])

            # ---- causal depthwise conv as 4 accumulated diagonal matmuls
            for cb in range(NCB):
                for n in range(NT):
                    pt = psum.tile([128, 512], F32, tag="mm")
                    for tap in range(D_CONV):
                        nc.tensor.matmul(
                            pt[:], cdiag[tap][cb][:],
                            xi_pad[cb][:, n * 512 + tap:n * 512 + tap + 512],
                            start=(tap == 0), stop=(tap == D_CONV - 1))
                    nc.scalar.activation(
                        xc_t[cb][:, n * 512:(n + 1) * 512], pt[:], AF.Silu,
                        bias=cb_t[:, cb:cb + 1])
                nc.sync.dma_start(out=xc_out[cb * 128:(cb + 1) * 128, :],
                                  in_=xc_t[cb][:])

            # ---- x_proj partial: dbc = wxpT.T @ xc   [96, L]
            for n in range(NT):
                pt = psum.tile([96, 512], F32, tag="xp")
                for kc in range(NCB):
                    nc.tensor.matmul(
                        pt[:], wxp_t[kc][:],
                        xc_t[kc][:, n * 512:(n + 1) * 512],
                        start=(kc == 0), stop=(kc == NCB - 1))
                dt = work.tile([96, 512], F32, tag="dbc")
                nc.scalar.activation(dt[:], pt[:], AF.Copy)
                nc.sync.dma_start(out=dbc_out[:, n * 512:(n + 1) * 512],
                                  in_=dt[:])
    nc.compile()
    return nc


def _build_b():
    nc = bacc.Bacc("TRN2", target_bir_lowering=False, debug=False,
                   num_devices=NCORES)
    xc_in = nc.dram_tensor("xc", [CH, L], BF16, kind="ExternalInput").ap()
    sres_in = nc.dram_tensor("sres", [CH, L], BF16, kind="ExternalInput").ap()
    dtr = nc.dram_tensor("dtr", [DT_RANK, L], F32, kind="ExternalInput").ap()
    brep = nc.dram_tensor("brep", [D_STATE * 128, L], BF16,
                          kind="ExternalInput").ap()
    crep = nc.dram_tensor("crep", [D_STATE * 128, L], BF16,
                          kind="ExternalInput").ap()
    wdtT = nc.dram_tensor("wdtT", [DT_RANK, CH], F32, kind="ExternalInput").ap()
    dtb = nc.dram_tensor("dtb", [128, NCB], F32, kind="ExternalInput").ap()
    woutT = nc.dram_tensor("woutT", [CH, D_MODEL], BF16,
                           kind="ExternalInput").ap()
    acol = nc.dram_tensor("acol", [128, D_STATE * NCB], F32,
                          kind="ExternalInput").ap()
    dcol = nc.dram_tensor("dcol", [128, NCB], F32, kind="ExternalInput").ap()
    ident = nc.dram_tensor("ident", [128, 128], BF16, kind="ExternalInput").ap()

    outp = nc.dram_tensor("outp", [D_MODEL, L], F32, kind="ExternalOutput").ap()

    with TileContext(nc) as tc:
        with (
            tc.tile_pool(name="const", bufs=1) as const,
            tc.tile_pool(name="bc", bufs=4) as bcpool,
            tc.tile_pool(name="dap", bufs=4) as dapool,
            tc.tile_pool(name="work", bufs=3) as work,
        ):
            dtr_t = const.tile([DT_RANK, L], F32, tag="dtr")
            nc.sync.dma_start(out=dtr_t[:], in_=dtr[:])
            wdt_t = const.tile([DT_RANK, CH], F32, tag="wdt")
            nc.sync.dma_start(out=wdt_t[:], in_=wdtT[:])
            dtb_t = const.tile([128, NCB], F32, tag="dtb")
            nc.sync.dma_start(out=dtb_t[:], in_=dtb[:])
            acol_t = const.tile([128, D_STATE * NCB], F32, tag="acol")
            nc.sync.dma_start(out=acol_t[:], in_=acol[:])
            dcol_t = const.tile([128, NCB], F32, tag="dcol")
            nc.sync.dma_start(out=dcol_t[:], in_=dcol[:])
            id_t = const.tile([128, 128], BF16, tag="ident")
            nc.sync.dma_start(out=id_t[:], in_=ident[:])
            xc_t, sres_t = [], []
            for cb in range(NCB):
                t = const.tile([128, L], BF16, tag=f"xc{cb}")
                nc.sync.dma_start(out=t[:], in_=xc_in[cb * 128:(cb + 1) * 128, :])
                xc_t.append(t)
                t = const.tile([128, L], BF16, tag=f"sr{cb}")
                nc.sync.dma_start(out=t[:], in_=sres_in[cb * 128:(cb + 1) * 128, :])
                sres_t.append(t)
            wout_t = []
            for kc in range(NCB):
                t = const.tile([128, D_MODEL], BF16, tag=f"wo{kc}")
                nc.sync.dma_start(out=t[:], in_=woutT[kc * 128:(kc + 1) * 128, :])
                wout_t.append(t)

            # ---- dt_proj + softplus -> delta [CH, L] fp32
            delta_t = []
            with tc.tile_pool(name="psum1", bufs=4, space="PSUM") as psum1:
              for m in range(NCB):
                dt = const.tile([128, L], F32, tag=f"dl{m}")
                ets = []
                for n in range(NT):
                    pt = psum1.tile([128, 512], F32, tag="mm")
                    nc.tensor.matmul(pt[:], wdt_t[:, m * 128:(m + 1) * 128],
                                     dtr_t[:, n * 512:(n + 1) * 512],
                                     start=True, stop=True)
                    # softplus(z) = ln(exp(z) + 1); batching the Exps then the
                    # Lns avoids ACT-table ping-pong
                    et = work.tile([128, 512], F32, tag="spe", bufs=4,
                                   name=f"spe{m}_{n}")
                    nc.scalar.activation(et[:], pt[:], AF.Exp,
                                         bias=dtb_t[:, m:m + 1])
                    ets.append(et)
                for n in range(NT):
                    nc.scalar.activation(dt[:, n * 512:(n + 1) * 512], ets[n][:],
                                         AF.Ln, bias=1.0)
                delta_t.append(dt)
              # (psum1 released before the scan's accumulator pool opens)

            # ---- u = delta * xc (bf16)
            u_t = []
            for cb in range(NCB):
                ut = const.tile([128, L], BF16, tag=f"u{cb}")
                nc.vector.tensor_mul(ut[:], delta_t[cb][:], xc_t[cb][:])
                u_t.append(ut)

            # ---- the scan: per (state, channel-block); the 16 C-weighted
            # state contributions are summed on the PE via identity-matmul
            # accumulation into PSUM (fp32).  Two half-passes of 2 channel
            # blocks each so 2x[128,2048] fp32 accumulators fill all 8 banks.
            y_t = [None] * NCB
            for half in range(2):
                cbs = [2 * half, 2 * half + 1]
                with tc.tile_pool(name=f"accp{half}", bufs=1,
                                  space="PSUM") as accpool:
                    accp = {}
                    for cb in cbs:
                        accp[cb] = accpool.tile([128, L], F32, tag=f"ac{cb}",
                                                name=f"accp{cb}")
                    for s in range(D_STATE):
                        br = bcpool.tile([128, L], BF16, tag="brep")
                        nc.sync.dma_start(out=br[:],
                                          in_=brep[s * 128:(s + 1) * 128, :])
                        cr = bcpool.tile([128, L], BF16, tag="crep")
                        nc.sync.dma_start(out=cr[:],
                                          in_=crep[s * 128:(s + 1) * 128, :])
                        for cb in cbs:
                            dA = dapool.tile([128, L], BF16, tag="dA")
                            nc.scalar.activation(dA[:], delta_t[cb][:], AF.Exp,
                                                 scale=acol_t[:, s * NCB + cb:
                                                              s * NCB + cb + 1])
                            bu = work.tile([128, L], BF16, tag="bu")
                            nc.vector.tensor_mul(bu[:], u_t[cb][:], br[:])
                            h = work.tile([128, L], BF16, tag="h")
                            nc.vector.tensor_tensor_scan(h[:], dA[:], bu[:], 0.0,
                                                         OP.mult, OP.add)
                            hc = work.tile([128, L], BF16, tag="hc")
                            nc.vector.tensor_mul(hc[:], h[:], cr[:])
                            for n in range(NT):
                                nc.tensor.matmul(
                                    accp[cb][:, n * 512:(n + 1) * 512],
                                    id_t[:],
                                    hc[:, n * 512:(n + 1) * 512],
                                    start=(s == 0), stop=(s == D_STATE - 1))
                    # ---- y = (acc + xc * D) * sres; y overwrites the spent
                    # xc tile (WAR handled by tile dep tracking)
                    for cb in cbs:
                        for n in range(NT):
                            sl = slice(n * 512, (n + 1) * 512)
                            t1 = work.tile([128, 512], BF16, tag="t1")
                            nc.vector.scalar_tensor_tensor(
                                t1[:], xc_t[cb][:, sl], dcol_t[:, cb:cb + 1],
                                accp[cb][:, sl], OP.mult, OP.add)
                            nc.vector.tensor_mul(xc_t[cb][:, sl], t1[:],
                                                 sres_t[cb][:, sl])
                        y_t[cb] = xc_t[cb]

            # ---- out_proj partial: outp = woutT.T @ y  [D_MODEL, L]
            with tc.tile_pool(name="psum2", bufs=4, space="PSUM") as psum2:
              for n in range(NT):
                for m in range(D_MODEL // 128):
                    pt = psum2.tile([128, 512], F32, tag="mm")
                    for kc in range(NCB):
                        nc.tensor.matmul(pt[:],
                                         wout_t[kc][:, m * 128:(m + 1) * 128],
                                         y_t[kc][:, n * 512:(n + 1) * 512],
                                         start=(kc == 0), stop=(kc == NCB - 1))
                    ot = work.tile([128, 512], F32, tag="ot")
                    nc.scalar.activation(ot[:], pt[:], AF.Copy)
                    nc.sync.dma_start(
                        out=outp[m * 128:(m + 1) * 128, n * 512:(n + 1) * 512],
                        in_=ot[:])
              # end psum2
    nc.compile()
    return nc


def _bf(a):
    return np.ascontiguousarray(a).astype(ml_dtypes.bfloat16)


def _f32(a):
    return np.ascontiguousarray(a, dtype=np.float32)


def kernel(x, in_proj_w, conv_w, conv_b, x_proj_w, dt_proj_w, dt_proj_b,
           A_log, D, out_proj_w):
    if "a" not in _CACHE:
        _CACHE["a"] = _build_a()
    if "b" not in _CACHE:
        _CACHE["b"] = _build_b()
    nca, ncb = _CACHE["a"], _CACHE["b"]

    A = -np.exp(np.asarray(A_log, np.float32))          # [D_INNER, D_STATE]
    x = np.asarray(x, np.float32)

    core_bq = [(c // 4, c % 4) for c in range(NCORES)]

    # ---------------- kernel A inputs
    xTb = [_bf(x[b].T) for b in range(B)]
    in_maps = []
    for b, q in core_bq:
        sl = slice(q * CH, (q + 1) * CH)
        w1 = np.concatenate([in_proj_w[sl], in_proj_w[D_INNER + q * CH:
                                                      D_INNER + (q + 1) * CH]], 0)
        cw = conv_w[sl, 0, :]                            # [CH, 4]
        cd = np.zeros((D_CONV * NCB * 128, 128), np.float32)
        for tap in range(D_CONV):
            for cb in range(NCB):
                blk = cd[(tap * NCB + cb) * 128:(tap * NCB + cb + 1) * 128]
                np.fill_diagonal(blk, cw[cb * 128:(cb + 1) * 128, tap])
        in_maps.append({
            "xT": xTb[b],
            "w1t": _bf(w1.T),
            "convdiag": _bf(cd),
            "convb": _f32(conv_b[sl].reshape(NCB, 128).T),
            "wxpT": _bf(x_proj_w[:, sl].T),
        })
    ra = run_bass_kernel_spmd(nca, in_maps, list(range(NCORES)))

    # ---------------- host exchange
    dbc = [None, None]
    for b in range(B):
        dbc[b] = sum(np.asarray(ra.results[4 * b + q]["dbc"], np.float32)
                     for q in range(4))
    in_maps_b = []
    breps, creps = [], []
    for b in range(B):
        Bm = dbc[b][DT_RANK:DT_RANK + D_STATE]           # [16, L]
        Cm = dbc[b][DT_RANK + D_STATE:]
        breps.append(_bf(np.repeat(Bm, 128, axis=0)))
        creps.append(_bf(np.repeat(Cm, 128, axis=0)))
    for c, (b, q) in enumerate(core_bq):
        sl = slice(q * CH, (q + 1) * CH)
        acol = np.zeros((128, D_STATE * NCB), np.float32)
        for s in range(D_STATE):
            for cb in range(NCB):
                acol[:, s * NCB + cb] = A[q * CH + cb * 128:
                                          q * CH + (cb + 1) * 128, s]
        in_maps_b.append({
            "xc": ra.results[c]["xc"],
            "sres": ra.results[c]["sres"],
            "dtr": _f32(dbc[b][:DT_RANK]),
            "brep": breps[b],
            "crep": creps[b],
            "wdtT": _f32(dt_proj_w[sl].T),
            "dtb": _f32(dt_proj_b[sl].reshape(NCB, 128).T),
            "woutT": _bf(out_proj_w[:, sl].T),
            "acol": acol,
            "dcol": _f32(D[sl].reshape(NCB, 128).T),
            "ident": _bf(np.eye(128, dtype=np.float32)),
        })
    rb = run_bass_kernel_spmd(ncb, in_maps_b, list(range(NCORES)))

    out = np.zeros((B, L, D_MODEL), np.float32)
    for b in range(B):
        acc = sum(np.asarray(rb.results[4 * b + q]["outp"], np.float32)
                  for q in range(4))
        out[b] = acc.T
    return out



# revision 9
# speedup vs baseline: 1.0491x; 1.0491x over previous
"""Trainium2 Bass kernel for a minimal Mamba layer (B=2, L=2048, d_model=1024,
d_inner=2048, d_state=16, d_conv=4, dt_rank=64) on 8 NeuronCores.

Sharding: core = (batch, d_inner-quarter).  Cores 0-3 handle batch 0, cores
4-7 batch 1; within a batch group each core owns 512 d_inner channels.

Two SPMD kernels with a tiny host exchange between them:
  A: in_proj (own rows) + causal depthwise conv (as 4 PSUM-accumulated
     diagonal matmuls) + silu + x_proj partial (own-channel contraction).
  host: sum the 4 partial dbc's per batch (96x2048 each), build broadcast
     tiles for B/C rows.
  B: dt_proj + softplus, then per (state, ch-block): dA = exp(A*delta) on
     ScalarE, Bu on VectorE, the SSM recurrence via the hardware
     tensor_tensor_scan, y accumulation, gating, out_proj partial.
  host: sum the 4 partial outputs per batch.
"""

import sys

if "/opt/trn_rl_repo" not in sys.path:
    sys.path.insert(0, "/opt/trn_rl_repo")

import numpy as np
import ml_dtypes

import concourse.bass as bass
from concourse import bacc, mybir
from concourse.bass_utils import run_bass_kernel_spmd
from concourse.tile import TileContext

F32 = mybir.dt.float32
BF16 = mybir.dt.bfloat16
AF = mybir.ActivationFunctionType
OP = mybir.AluOpType


# ---------------------------------------------------------------------------
# Custom DVE op: dual interleaved affine scan (2 independent first-order
# recurrences alternating per element at 1 elem/cycle; the stock
# TENSOR_TENSOR_SCAN runs 1 recurrence at 2 cycles/elem because the
# mult->add carried chain spans two pipeline stages — interleaving two
# sequences hides that feedback latency).  Element e's MULT at block 0 reads
# NEXT_ALU_OUT_B = block 1's b-result-flop from the previous cycle, i.e. the
# ADD result (state) of element e-2: the same sequence.
# ---------------------------------------------------------------------------

def _dual_scan_reference(in0, in1, c0, c1, c2):
    a = np.asarray(in0, np.float32).reshape(in0.shape[0], -1)
    b = np.asarray(in1, np.float32).reshape(in1.shape[0], -1)
    out = np.empty_like(a)
    s = [np.broadcast_to(np.asarray(c0, np.float32), (a.shape[0],)).astype(np.float32).copy(),
         np.broadcast_to(np.asarray(c1, np.float32), (a.shape[0],)).astype(np.float32).copy()]
    for k in range(a.shape[1]):
        j = k & 1
        s[j] = a[:, k] * s[j] + b[:, k]
        out[:, k] = s[j]
    return out


def _build_dual_scan_uops():
    from concourse.dve_uop import (
        UopConfig, InpSel, OutSel, OutPath, AluOp, AluInp, Trigger,
    )

    def mk_init(init_const, next_idx):
        u = UopConfig()
        u.enable_input(InpSel.SRC_0, 1)    # PREV_DELAY_0 = decay a
        u.enable_input(InpSel.SRC_1, 2)    # PREV_DELAY_1 = additive b
        u.enable_input(init_const, 3)      # PREV_DELAY_2 = initial state
        u.require_inp0 = 1
        u.require_inp1 = 1
        u.repeat_count = 1
        u.trigger = (Trigger.COUNT, Trigger.NONE, Trigger.NONE)
        u.next_uop = (next_idx, 0, 0)
        dp = u.datapath_config
        dp[0].enable_alu(AluOp.MULTIPLY, AluInp.PREV_DELAY_0, AluInp.PREV_DELAY_2)
        dp[0].pass_through_delay(1)
        dp[1].enable_alu(AluOp.ADD, AluInp.PREV_ALU_OUT, AluInp.PREV_DELAY_1)
        dp[1].alu_out_b_enable = 1
        for k in range(2, 8):
            dp[k].pass_through_alu()
        u.enable_output(OutSel.ALU_OUT, OutPath.WR0_LO)
        return u

    us = UopConfig()
    us.enable_input(InpSel.SRC_0, 1)
    us.enable_input(InpSel.SRC_1, 2)
    us.require_inp0 = 1
    us.require_inp1 = 1
    us.trigger = (Trigger.SRC_TENSOR_DONE, Trigger.NONE, Trigger.NONE)
    us.next_uop = (0, 0, 0)
    dp = us.datapath_config
    dp[0].enable_alu(AluOp.MULTIPLY, AluInp.PREV_DELAY_0, AluInp.NEXT_ALU_OUT_B)
    dp[0].pass_through_delay(1)
    dp[1].enable_alu(AluOp.ADD, AluInp.PREV_ALU_OUT, AluInp.PREV_DELAY_1)
    dp[1].alu_out_b_enable = 1
    for k in range(2, 8):
        dp[k].pass_through_alu()
    us.enable_output(OutSel.ALU_OUT, OutPath.WR0_LO)
    return [mk_init(InpSel.CONST_0, 1), mk_init(InpSel.CONST_1, 2), us]


class _DualScanOp:
    name = "DUAL_AFFINE_SCAN_ANT"
    subdim = False
    perf_en = {}

    def __init__(self):
        from concourse.dve_spec import Spec, Src0, Src1
        self.spec = Spec(body=Src0 * Src1, reference=_dual_scan_reference)
        self._cache = {}

    def compile(self, ver):
        if ver not in self._cache:
            from concourse.dve_uop import DveOpSpec
            from concourse.dve_ops import get_dve_sub_opcode
            self._cache[ver] = DveOpSpec(
                name=self.name,
                opcode=get_dve_sub_opcode(self.name),
                uops=_build_dual_scan_uops(),
                rd1_en=True,
            )
        return self._cache[ver]


_DS = {}


def _install_dual_scan():
    if "op" in _DS:
        return _DS["op"]
    import concourse.dve_ops as dve_ops
    op = _DualScanOp()
    if op.name not in dve_ops._SUB_OPCODE_FOR_NAME:
        row = dve_ops._CUSTOM_DVE_ROW_BASE + len(dve_ops.OPS)
        assert row < 0x20
        dve_ops.OPS.append(op)
        dve_ops._SUB_OPCODE_FOR_NAME[op.name] = row
        dve_ops.CUSTOM_DVE_SPECS[op.name] = op.spec
    _DS["op"] = op
    return op


def dual_scan(nc, out, in0, in1, init0=0.0, init1=0.0):
    """out/in0/in1: [128, T, 2] APs (T timesteps x 2 interleaved sequences)."""
    op = _install_dual_scan()
    return nc.vector._custom_dve(op, out=out, in0=in0, in1=in1,
                                 s0=init0, s1=init1)

D_MODEL = 1024
D_STATE = 16
D_CONV = 4
D_INNER = 2048
DT_RANK = 64
B = 2
L = 2048
NCORES = 8
CH = D_INNER // 4          # 512 channels per core
NCB = CH // 128            # 4 channel blocks of 128
NT = L // 512              # 4 token tiles of 512
KM = D_MODEL // 128        # 8 k tiles for in_proj

_CACHE = {}


def _build_a():
    nc = bacc.Bacc("TRN2", target_bir_lowering=False, debug=False,
                   num_devices=NCORES)
    xT = nc.dram_tensor("xT", [D_MODEL, L], BF16, kind="ExternalInput").ap()
    w1t = nc.dram_tensor("w1t", [D_MODEL, 2 * CH], BF16, kind="ExternalInput").ap()
    convdiag = nc.dram_tensor("convdiag", [D_CONV * NCB * 128, 128], BF16,
                              kind="ExternalInput").ap()
    convb = nc.dram_tensor("convb", [128, NCB], F32, kind="ExternalInput").ap()
    wxpT = nc.dram_tensor("wxpT", [CH, 96], BF16, kind="ExternalInput").ap()

    xc_out = nc.dram_tensor("xc", [CH, L], BF16, kind="ExternalOutput").ap()
    sres_out = nc.dram_tensor("sres", [CH, L], BF16, kind="ExternalOutput").ap()
    dbc_out = nc.dram_tensor("dbc", [96, L], F32, kind="ExternalOutput").ap()

    with TileContext(nc) as tc:
        with (
            tc.tile_pool(name="const", bufs=1) as const,
            tc.tile_pool(name="psum", bufs=4, space="PSUM") as psum,
            tc.tile_pool(name="work", bufs=3) as work,
        ):
            xT_t, w1_t = [], []
            for k in range(KM):
                t = const.tile([128, L], BF16, tag=f"xT{k}")
                nc.sync.dma_start(out=t[:], in_=xT[k * 128:(k + 1) * 128, :])
                xT_t.append(t)
                t = const.tile([128, 2 * CH], BF16, tag=f"w1{k}", name=f"w1{k}")
                nc.sync.dma_start(out=t[:], in_=w1t[k * 128:(k + 1) * 128, :])
                w1_t.append(t)
            cdiag = []
            for tap in range(D_CONV):
                row = []
                for cb in range(NCB):
                    t = const.tile([128, 128], BF16, tag=f"cd{tap}_{cb}")
                    off = (tap * NCB + cb) * 128
                    nc.sync.dma_start(out=t[:], in_=convdiag[off:off + 128, :])
                    row.append(t)
                cdiag.append(row)
            cb_t = const.tile([128, NCB], F32, tag="convb")
            nc.sync.dma_start(out=cb_t[:], in_=convb[:])
            wxp_t = []
            for kc in range(NCB):
                t = const.tile([128, 96], BF16, tag=f"wxp{kc}")
                nc.sync.dma_start(out=t[:], in_=wxpT[kc * 128:(kc + 1) * 128, :])
                wxp_t.append(t)

            # xi (post in_proj, pre conv): padded with 3 zero columns in front
            xi_pad = []
            for cb in range(NCB):
                t = const.tile([128, L + D_CONV - 1], BF16, tag=f"xip{cb}")
                nc.vector.memset(t[:, 0:D_CONV - 1], 0.0)
                xi_pad.append(t)
            xc_t = [const.tile([128, L], BF16, tag=f"xc{cb}", name=f"xc{cb}") for cb in range(NCB)]

            # ---- in_proj: out rows 0..511 = xi slice, 512..1023 = res slice
            for n in range(NT):
                for m in range(2 * NCB):
                    pt = psum.tile([128, 512], F32, tag="mm")
                    for k in range(KM):
                        nc.tensor.matmul(
                            pt[:], w1_t[k][:, m * 128:(m + 1) * 128],
                            xT_t[k][:, n * 512:(n + 1) * 512],
                            start=(k == 0), stop=(k == KM - 1))
                    if m < NCB:
                        nc.scalar.activation(
                            xi_pad[m][:, D_CONV - 1 + n * 512:
                                      D_CONV - 1 + (n + 1) * 512],
                            pt[:], AF.Copy)
                    else:
                        st = work.tile([128, 512], BF16, tag="sres")
                        nc.scalar.activation(st[:], pt[:], AF.Silu)
                        nc.sync.dma_start(
                            out=sres_out[(m - NCB) * 128:(m - NCB + 1) * 128,
                                         n * 512:(n + 1) * 512],
                            in_=st[:])

            # ---- causal depthwise conv as 4 accumulated diagonal matmuls
            for cb in range(NCB):
                for n in range(NT):
                    pt = psum.tile([128, 512], F32, tag="mm")
                    for tap in range(D_CONV):
                        nc.tensor.matmul(
                            pt[:], cdiag[tap][cb][:],
                            xi_pad[cb][:, n * 512 + tap:n * 512 + tap + 512],
                            start=(tap == 0), stop=(tap == D_CONV - 1))
                    nc.scalar.activation(
                        xc_t[cb][:, n * 512:(n + 1) * 512], pt[:], AF.Silu,
                        bias=cb_t[:, cb:cb + 1])
                nc.sync.dma_start(out=xc_out[cb * 128:(cb + 1) * 128, :],
                                  in_=xc_t[cb][:])

            # ---- x_proj partial: dbc = wxpT.T @ xc   [96, L]
            for n in range(NT):
                pt = psum.tile([96, 512], F32, tag="xp")
                for kc in range(NCB):
                    nc.tensor.matmul(
                        pt[:], wxp_t[kc][:],
                        xc_t[kc][:, n * 512:(n + 1) * 512],
                        start=(kc == 0), stop=(kc == NCB - 1))
                dt = work.tile([96, 512], F32, tag="dbc", bufs=2)
                nc.scalar.activation(dt[:], pt[:], AF.Copy)
                nc.sync.dma_start(out=dbc_out[:, n * 512:(n + 1) * 512],
                                  in_=dt[:])
    nc.compile()
    return nc


def _build_b():
    nc = bacc.Bacc("TRN2", target_bir_lowering=False, debug=False,
                   num_devices=NCORES)
    xc_in = nc.dram_tensor("xc", [CH, L], BF16, kind="ExternalInput").ap()
    sres_in = nc.dram_tensor("sres", [CH, L], BF16, kind="ExternalInput").ap()
    dtr = nc.dram_tensor("dtr", [DT_RANK, L], F32, kind="ExternalInput").ap()
    brep = nc.dram_tensor("brep", [D_STATE // 2 * 128, 2 * L], BF16,
                          kind="ExternalInput").ap()
    crep = nc.dram_tensor("crep", [D_STATE // 2 * 128, 2 * L], BF16,
                          kind="ExternalInput").ap()
    wdtT = nc.dram_tensor("wdtT", [DT_RANK, CH], F32, kind="ExternalInput").ap()
    dtb = nc.dram_tensor("dtb", [128, NCB], F32, kind="ExternalInput").ap()
    woutT = nc.dram_tensor("woutT", [CH, D_MODEL], BF16,
                           kind="ExternalInput").ap()
    acol = nc.dram_tensor("acol", [128, D_STATE * NCB], F32,
                          kind="ExternalInput").ap()
    dcol = nc.dram_tensor("dcol", [128, NCB], F32, kind="ExternalInput").ap()
    ident = nc.dram_tensor("ident", [128, 128], BF16, kind="ExternalInput").ap()

    outp = nc.dram_tensor("outp", [D_MODEL, L], F32, kind="ExternalOutput").ap()

    with TileContext(nc) as tc:
        with (
            tc.tile_pool(name="const", bufs=1) as const,
            tc.tile_pool(name="bc", bufs=2) as bcpool,
            tc.tile_pool(name="dap", bufs=2) as dapool,
            tc.tile_pool(name="work", bufs=3) as work,
        ):
            dtr_t = const.tile([DT_RANK, L], F32, tag="dtr")
            nc.sync.dma_start(out=dtr_t[:], in_=dtr[:])
            wdt_t = const.tile([DT_RANK, CH], F32, tag="wdt")
            nc.sync.dma_start(out=wdt_t[:], in_=wdtT[:])
            dtb_t = const.tile([128, NCB], F32, tag="dtb")
            nc.sync.dma_start(out=dtb_t[:], in_=dtb[:])
            acol_t = const.tile([128, D_STATE * NCB], F32, tag="acol")
            nc.sync.dma_start(out=acol_t[:], in_=acol[:])
            dcol_t = const.tile([128, NCB], F32, tag="dcol")
            nc.sync.dma_start(out=dcol_t[:], in_=dcol[:])
            id_t = const.tile([128, 128], BF16, tag="ident")
            nc.sync.dma_start(out=id_t[:], in_=ident[:])
            xc_t, sres_t = [], []
            for cb in range(NCB):
                t = const.tile([128, L], BF16, tag=f"xc{cb}")
                nc.sync.dma_start(out=t[:], in_=xc_in[cb * 128:(cb + 1) * 128, :])
                xc_t.append(t)
                t = const.tile([128, L], BF16, tag=f"sr{cb}")
                nc.sync.dma_start(out=t[:], in_=sres_in[cb * 128:(cb + 1) * 128, :])
                sres_t.append(t)
            wout_t = []
            for kc in range(NCB):
                t = const.tile([128, D_MODEL], BF16, tag=f"wo{kc}")
                nc.sync.dma_start(out=t[:], in_=woutT[kc * 128:(kc + 1) * 128, :])
                wout_t.append(t)

            # ---- dt_proj + softplus -> delta [CH, L] fp32
            delta_t = []
            with tc.tile_pool(name="psum1", bufs=4, space="PSUM") as psum1:
              for m in range(NCB):
                dt = const.tile([128, L], BF16, tag=f"dl{m}")
                ets = []
                for n in range(NT):
                    pt = psum1.tile([128, 512], F32, tag="mm")
                    nc.tensor.matmul(pt[:], wdt_t[:, m * 128:(m + 1) * 128],
                                     dtr_t[:, n * 512:(n + 1) * 512],
                                     start=True, stop=True)
                    # softplus(z) = ln(exp(z) + 1); batching the Exps then the
                    # Lns avoids ACT-table ping-pong
                    et = work.tile([128, 512], F32, tag="spe", bufs=2,
                                   name=f"spe{m}_{n}")
                    nc.scalar.activation(et[:], pt[:], AF.Exp,
                                         bias=dtb_t[:, m:m + 1])
                    ets.append(et)
                for n in range(NT):
                    nc.scalar.activation(dt[:, n * 512:(n + 1) * 512], ets[n][:],
                                         AF.Ln, bias=1.0)
                delta_t.append(dt)
              # (psum1 released before the scan's accumulator pool opens)

            # ---- u = delta * xc (bf16), then element-interleaved duplicate
            # u_rep[:, 2t] = u_rep[:, 2t+1] = u[:, t] (built on ScalarE)
            u_rep = []
            for cb in range(NCB):
                ut = work.tile([128, L], BF16, tag="bu", bufs=2, name=f"ur{cb}")
                nc.vector.tensor_mul(ut[:], delta_t[cb][:], xc_t[cb][:])
                urt = const.tile([128, 2 * L], BF16, tag=f"u{cb}")
                nc.scalar.activation(urt[:, 0:2 * L:2], ut[:], AF.Copy)
                nc.scalar.activation(urt[:, 1:2 * L:2], ut[:], AF.Copy)
                u_rep.append(urt)

            # ---- the scan: per (state-pair, channel-block).  Each dual_scan
            # instruction runs TWO states' recurrences interleaved at
            # 1 elem/cycle (vs 2 for the stock scan).  Tiles hold the two
            # states as contiguous planes [128, 2*L]; only the scan reads
            # them through an interleaving [128, L, 2] access pattern, so
            # all muls/exps stay contiguous (DVE 2x mode).
            NP = D_STATE // 2
            y_t = [None] * NCB
            for half in range(2):
                cbs = [2 * half, 2 * half + 1]
                with tc.tile_pool(name=f"accp{half}", bufs=1,
                                  space="PSUM") as accpool:
                    accp = {}
                    for cb in cbs:
                        accp[cb] = accpool.tile([128, L], F32, tag=f"ac{cb}",
                                                name=f"accp{cb}")
                    for p in range(NP):
                        s0, s1 = 2 * p, 2 * p + 1
                        brp = bcpool.tile([128, 2 * L], BF16, tag="brp")
                        nc.sync.dma_start(out=brp[:],
                                          in_=brep[p * 128:(p + 1) * 128, :])
                        crp = bcpool.tile([128, 2 * L], BF16, tag="crp")
                        nc.sync.dma_start(out=crp[:],
                                          in_=crep[p * 128:(p + 1) * 128, :])
                        for cb in cbs:
                            dA = dapool.tile([128, 2 * L], BF16, tag="dA")
                            nc.scalar.activation(
                                dA[:, 0:2 * L:2], delta_t[cb][:], AF.Exp,
                                scale=acol_t[:, s0 * NCB + cb:s0 * NCB + cb + 1])
                            nc.scalar.activation(
                                dA[:, 1:2 * L:2], delta_t[cb][:], AF.Exp,
                                scale=acol_t[:, s1 * NCB + cb:s1 * NCB + cb + 1])
                            bu = work.tile([128, 2 * L], BF16, tag="bu", bufs=2)
                            nc.vector.tensor_mul(bu[:], u_rep[cb][:], brp[:])
                            h = work.tile([128, 2 * L], BF16, tag="h", bufs=2)
                            dual_scan(nc, h[:], dA[:], bu[:])
                            hc = work.tile([128, 2 * L], BF16, tag="hc", bufs=2)
                            eng = nc.gpsimd if (p * NCB + cb) % 3 == 2 else nc.vector
                            eng.tensor_mul(hc[:], h[:], crp[:])
                            hcv = [hc[:, 0:2 * L:2], hc[:, 1:2 * L:2]]
                            for j in range(2):
                                for n in range(NT):
                                    nc.tensor.matmul(
                                        accp[cb][:, n * 512:(n + 1) * 512],
                                        id_t[:],
                                        hcv[j][:, n * 512:(n + 1) * 512],
                                        start=(p == 0 and j == 0),
                                        stop=(p == NP - 1 and j == 1))
                    # ---- y = (acc + xc * D) * sres; y overwrites the spent
                    # xc tile (WAR handled by tile dep tracking)
                    for cb in cbs:
                        for n in range(NT):
                            sl = slice(n * 512, (n + 1) * 512)
                            t1 = work.tile([128, 512], BF16, tag="t1", bufs=2)
                            nc.vector.scalar_tensor_tensor(
                                t1[:], xc_t[cb][:, sl], dcol_t[:, cb:cb + 1],
                                accp[cb][:, sl], OP.mult, OP.add)
                            nc.vector.tensor_mul(xc_t[cb][:, sl], t1[:],
                                                 sres_t[cb][:, sl])
                        y_t[cb] = xc_t[cb]

            # ---- out_proj partial: outp = woutT.T @ y  [D_MODEL, L]
            with tc.tile_pool(name="psum2", bufs=4, space="PSUM") as psum2:
              for n in range(NT):
                for m in range(D_MODEL // 128):
                    pt = psum2.tile([128, 512], F32, tag="mm")
                    for kc in range(NCB):
                        nc.tensor.matmul(pt[:],
                                         wout_t[kc][:, m * 128:(m + 1) * 128],
                                         y_t[kc][:, n * 512:(n + 1) * 512],
                                         start=(kc == 0), stop=(kc == NCB - 1))
                    ot = work.tile([128, 512], F32, tag="ot", bufs=2)
                    nc.scalar.activation(ot[:], pt[:], AF.Copy)
                    nc.sync.dma_start(
                        out=outp[m * 128:(m + 1) * 128, n * 512:(n + 1) * 512],
                        in_=ot[:])
              # end psum2
    nc.compile()
    return nc


def _bf(a):
    return np.ascontiguousarray(a).astype(ml_dtypes.bfloat16)


def _f32(a):
    return np.ascontiguousarray(a, dtype=np.float32)


def kernel(x, in_proj_w, conv_w, conv_b, x_proj_w, dt_proj_w, dt_proj_b,
           A_log, D, out_proj_w):
    if "a" not in _CACHE:
        _CACHE["a"] = _build_a()
    if "b" not in _CACHE:
        _CACHE["b"] = _build_b()
    nca, ncb = _CACHE["a"], _CACHE["b"]

    A = -np.exp(np.asarray(A_log, np.float32))          # [D_INNER, D_STATE]
    x = np.asarray(x, np.float32)

    core_bq = [(c // 4, c % 4) for c in range(NCORES)]

    # ---------------- kernel A inputs
    xTb = [_bf(x[b].T) for b in range(B)]
    in_maps = []
    for b, q in core_bq:
        sl = slice(q * CH, (q + 1) * CH)
        w1 = np.concatenate([in_proj_w[sl], in_proj_w[D_INNER + q * CH:
                                                      D_INNER + (q + 1) * CH]], 0)
        cw = conv_w[sl, 0, :]                            # [CH, 4]
        cd = np.zeros((D_CONV * NCB * 128, 128), np.float32)
        for tap in range(D_CONV):
            for cb in range(NCB):
                blk = cd[(tap * NCB + cb) * 128:(tap * NCB + cb + 1) * 128]
                np.fill_diagonal(blk, cw[cb * 128:(cb + 1) * 128, tap])
        in_maps.append({
            "xT": xTb[b],
            "w1t": _bf(w1.T),
            "convdiag": _bf(cd),
            "convb": _f32(conv_b[sl].reshape(NCB, 128).T),
            "wxpT": _bf(x_proj_w[:, sl].T),
        })
    ra = run_bass_kernel_spmd(nca, in_maps, list(range(NCORES)))

    # ---------------- host exchange
    dbc = [None, None]
    for b in range(B):
        dbc[b] = sum(np.asarray(ra.results[4 * b + q]["dbc"], np.float32)
                     for q in range(4))
    in_maps_b = []
    breps, creps = [], []
    for b in range(B):
        Bm = dbc[b][DT_RANK:DT_RANK + D_STATE]           # [16, L]
        Cm = dbc[b][DT_RANK + D_STATE:]
        # pair-interleave: row p = [s_{2p}(t0), s_{2p+1}(t0), s_{2p}(t1), ...]
        Bi = np.stack([Bm[0::2], Bm[1::2]], axis=-1).reshape(D_STATE // 2, 2 * L)
        Ci = np.stack([Cm[0::2], Cm[1::2]], axis=-1).reshape(D_STATE // 2, 2 * L)
        breps.append(_bf(np.repeat(Bi, 128, axis=0)))
        creps.append(_bf(np.repeat(Ci, 128, axis=0)))
    for c, (b, q) in enumerate(core_bq):
        sl = slice(q * CH, (q + 1) * CH)
        acol = np.zeros((128, D_STATE * NCB), np.float32)
        for s in range(D_STATE):
            for cb in range(NCB):
                acol[:, s * NCB + cb] = A[q * CH + cb * 128:
                                          q * CH + (cb + 1) * 128, s]
        in_maps_b.append({
            "xc": ra.results[c]["xc"],
            "sres": ra.results[c]["sres"],
            "dtr": _f32(dbc[b][:DT_RANK]),
            "brep": breps[b],
            "crep": creps[b],
            "wdtT": _f32(dt_proj_w[sl].T),
            "dtb": _f32(dt_proj_b[sl].reshape(NCB, 128).T),
            "woutT": _bf(out_proj_w[:, sl].T),
            "acol": acol,
            "dcol": _f32(D[sl].reshape(NCB, 128).T),
            "ident": _bf(np.eye(128, dtype=np.float32)),
        })
    rb = run_bass_kernel_spmd(ncb, in_maps_b, list(range(NCORES)))

    out = np.zeros((B, L, D_MODEL), np.float32)
    for b in range(B):
        acc = sum(np.asarray(rb.results[4 * b + q]["outp"], np.float32)
                  for q in range(4))
        out[b] = acc.T
    return out

